# revision 1
# baseline (speedup 1.0000x reference)
"""Trainium2 Bass kernel for nn_Extract_HyperSpherePrototypes.

Computation (see reference):
  1. L2-normalize each pixel's feature vector over the channel dim F=256.
  2. Segment-sum normalized features by label into [C+1=20, F] prototypes.
  3. Drop void class, transpose to [F, 19], L2-normalize each column.

Sharding: data-parallel over batch (16 items / 8 cores = 2 per core).
Each core computes a local [20, 256] partial, AllReduce(sum) across the
8 cores, then every core normalizes columns and writes the full output.

Per-core layout: features[b] is loaded as four f-chunk tiles
[h=128; f=64, w=128] (partition = h, w innermost) so every DMA burst is a
contiguous 512B run (full line rate). The per-pixel inverse norm is folded
into a one-hot matrix M[h, w, c] = (label==c) * rsqrt(sum_f x^2), so the
segment-sum contracts h on the tensor engine. To keep the moving operand
at N=256 (float32r matmuls run 1 cycle/row there vs 4 for fp32), each
matmul packs QW=4 w-columns: lhsT stacks 4 w's masks in 32-partition
blocks (padded for PSUM alignment), rhs spans [64 f x 4 w]; the cross
(wl != wl') blocks land in distinct PSUM columns and are dropped by the
final diagonal-block combine. Set KERNEL_MM_DTYPE=f32 for full-precision
fp32 matmuls (~4x slower PE, ~3.4e-6 rel err vs ~1.4e-4 for f32r).
"""

import os

import numpy as np

import concourse.bass as bass
import concourse.bacc as bacc
from concourse import mybir
from concourse.bass_utils import run_bass_kernel_spmd
from concourse.tile import TileContext

F32 = mybir.dt.float32
F32R = mybir.dt.float32r
AX = mybir.AxisListType
OP = mybir.AluOpType
ACT_FN = mybir.ActivationFunctionType

NCORES = 8
B_TOT = 16
BPC = B_TOT // NCORES  # batches per core
F = 256
H = 128
W = 128
C = 20  # 19 known + void
FC = 64  # f-chunk per tile
NFC = F // FC
WH = 64  # w-half for square scratch
QW = 4  # w-columns packed per matmul (lhsT = [h, QW*CP])
CP = 32  # class block padded to PSUM partition alignment
NQ = W // QW

EPS2 = 1e-24  # matches max(norm, 1e-12) in the reference

_NO_CC = bool(int(os.environ.get("KERNEL_NO_CC", "0")))
_MM_F32 = os.environ.get("KERNEL_MM_DTYPE", "f32r") == "f32"


def build_nc():
    mm_dt = F32 if _MM_F32 else F32R
    nc = bacc.Bacc("TRN2", target_bir_lowering=False)

    feats = nc.declare_dram_parameter("feats", [BPC, F, H, W], mm_dt, isOutput=False)
    labs = nc.declare_dram_parameter("labs", [BPC, H, W], F32, isOutput=False)
    out_d = nc.declare_dram_parameter("out", [F, C - 1], F32, isOutput=True)

    cc_in = nc.dram_tensor("cc_in", [C, F], F32)
    cc_out = nc.dram_tensor("cc_out", [C, F], F32, addr_space="Shared")

    with TileContext(nc) as tc:
        with (
            tc.tile_pool(name="consts", bufs=1) as consts,
            tc.tile_pool(name="xp", bufs=4) as xp,
            tc.tile_pool(name="sqp", bufs=1) as sqp,
            tc.tile_pool(name="mp", bufs=2) as mp,
            tc.tile_pool(name="normp", bufs=2) as normp,
            tc.tile_pool(name="finp", bufs=1) as finp,
            tc.tile_pool(name="psum", bufs=1, space="PSUM") as psum,
        ):
            iota_i = consts.tile([H, CP], mybir.dt.int32)
            nc.gpsimd.iota(iota_i, pattern=[[1, CP]], base=0, channel_multiplier=0)
            iota_sb = consts.tile([H, CP], F32)
            nc.vector.tensor_copy(iota_sb, iota_i)
            eps_sb = consts.tile([H, 1], F32)
            nc.vector.memset(eps_sb, EPS2)

            feats_ap = feats.ap()
            labs_ap = labs.ap()

            psq = []
            for fc in range(NFC):
                psq_t = psum.tile([QW * CP, FC * QW], F32, tag=f"ps{fc}")
                psq.append(psq_t)

            for b in range(BPC):
                lab_sb = normp.tile([H, W], F32)
                nc.sync.dma_start(out=lab_sb, in_=labs_ap[b])

                hfw = feats_ap[b].rearrange("f h w -> h f w")
                ssq4 = normp.tile([H, W, NFC], F32)
                xts = []
                for fc in range(NFC):
                    xt = xp.tile([H, FC, W], mm_dt)
                    nc.sync.dma_start(
                        out=xt, in_=hfw[:, fc * FC : (fc + 1) * FC, :]
                    )
                    xts.append(xt)
                    # sumsq over f per (h, w): square on ACT, reduce on DVE
                    for wh in range(W // WH):
                        sq = sqp.tile([H, FC, WH], F32)
                        src = xt[:, :, wh * WH : (wh + 1) * WH]
                        if not _MM_F32:
                            src = src.bitcast(F32)
                        nc.scalar.activation(out=sq, in_=src, func=ACT_FN.Square)
                        nc.vector.tensor_reduce(
                            out=ssq4[:, wh * WH : (wh + 1) * WH, fc],
                            in_=sq.rearrange("h f w -> h w f"),
                            axis=AX.X,
                            op=OP.add,
                        )
                ssq = normp.tile([H, W], F32)
                nc.vector.tensor_reduce(out=ssq, in_=ssq4, axis=AX.X, op=OP.add)
                nc.scalar.activation(out=ssq, in_=ssq, func=ACT_FN.Sqrt, bias=eps_sb[:])
                inv = normp.tile([H, W], F32)
                nc.vector.reciprocal(out=inv, in_=ssq)

                # M[h, w, c] = (iota_c == lab) * inv   (rounded to mm dtype);
                # c padded to CP=32 so PSUM class blocks are partition-aligned
                m_sb = mp.tile([H, W, CP], mm_dt)
                m_f32 = m_sb[:]
                nc.vector.tensor_tensor(
                    out=m_f32,
                    in0=bass.AP(
                        tensor=iota_sb[:].tensor,
                        offset=iota_sb[:].offset,
                        ap=[iota_sb[:].ap[0], [0, W], [1, CP]],
                    ),
                    in1=lab_sb[:].to_broadcast([H, W, CP]),
                    op=OP.is_equal,
                )
                nc.vector.tensor_tensor(
                    out=m_sb,
                    in0=m_f32,
                    in1=inv[:].to_broadcast([H, W, CP]),
                    op=OP.mult,
                )

                # segment-sum, QW w-columns per matmul:
                #   psq[fc][wl*C + c, f*QW + wl'] += sum_h M[h, q*QW+wl, c] X[h, f, q*QW+wl']
                # diagonal wl == wl' blocks are the real contributions.
                for fc in range(NFC):
                    for q in range(NQ):
                        nc.tensor.matmul(
                            out=psq[fc],
                            lhsT=m_sb[:, q * QW : (q + 1) * QW, :].rearrange(
                                "h w c -> h (w c)"
                            ),
                            rhs=xts[fc][:, :, q * QW : (q + 1) * QW],
                            start=(b == 0 and q == 0),
                            stop=(b == BPC - 1 and q == NQ - 1),
                        )

            # combine diagonal blocks: protos[c, fc*FC + f] = sum_wl psq[fc][wl*C+c, f*QW+wl]
            protos_sb = finp.tile([C, F], F32)
            for fc in range(NFC):
                pv = psq[fc][:].rearrange("m (f w) -> m f w", w=QW)
                dst = protos_sb[:, fc * FC : (fc + 1) * FC]
                nc.scalar.copy(out=dst, in_=pv[0:C, :, 0])
                for wl in range(1, QW):
                    nc.vector.tensor_add(
                        dst, dst, pv[wl * CP : wl * CP + C, :, wl]
                    )
            if not _NO_CC:
                nc.sync.dma_start(out=cc_in.ap(), in_=protos_sb)
                nc.gpsimd.collective_compute(
                    "AllReduce",
                    OP.add,
                    ins=[cc_in.ap().opt()],
                    outs=[cc_out.ap().opt()],
                    replica_groups=[list(range(NCORES))],
                )
                red_sb = finp.tile([C, F], F32)
                nc.sync.dma_start(out=red_sb, in_=cc_out.ap())
            else:
                red_sb = protos_sb

            # column norms (per class over F): pn2[c] = sum_f red[c,f]^2
            scr = finp.tile([C, F], F32)
            pn = finp.tile([C, 1], F32)
            nc.vector.tensor_mul(scr, red_sb, red_sb)
            nc.vector.tensor_reduce(out=pn, in_=scr, axis=AX.X, op=OP.add)
            nc.scalar.activation(out=pn, in_=pn, func=ACT_FN.Sqrt, bias=eps_sb[:C])
            pninv = finp.tile([C, 1], F32)
            nc.vector.reciprocal(out=pninv, in_=pn)
            nc.vector.tensor_scalar_mul(out=red_sb, in0=red_sb, scalar1=pninv)

            # transposed write: out[f, c] = red_sb[c, f]
            o_ap = out_d.ap()
            nc.sync.dma_start(
                out=bass.AP(
                    tensor=o_ap.tensor,
                    offset=o_ap.offset,
                    ap=[[1, C - 1], [C - 1, F]],
                ),
                in_=red_sb[0 : C - 1, :],
            )

    nc.compile()
    return nc


_NC_CACHE = None


def _get_nc():
    global _NC_CACHE
    if _NC_CACHE is None:
        _NC_CACHE = build_nc()
    return _NC_CACHE


def kernel(features: np.ndarray, labels: np.ndarray) -> np.ndarray:
    features = np.ascontiguousarray(np.asarray(features, dtype=np.float32))
    labs_f32 = np.asarray(labels, dtype=np.float32)  # values 0..19, exact in f32

    nc = _get_nc()
    in_maps = []
    for core in range(NCORES):
        in_maps.append(
            {
                "feats": features[core * BPC : (core + 1) * BPC],
                "labs": np.ascontiguousarray(labs_f32[core * BPC : (core + 1) * BPC]),
            }
        )
    res = run_bass_kernel_spmd(nc, in_maps, core_ids=list(range(NCORES)))
    return np.asarray(res.results[0]["out"], dtype=np.float32)



# revision 20
# speedup vs baseline: 1.5831x; 1.5831x over previous
"""Trainium2 Bass kernel for nn_Extract_HyperSpherePrototypes.

Computation (see reference):
  1. L2-normalize each pixel's feature vector over the channel dim F=256.
  2. Segment-sum normalized features by label into [C+1=20, F] prototypes.
  3. Drop void class, transpose to [F, 19], L2-normalize each column.

Sharding: data-parallel over batch (16 items / 8 cores = 2 per core).
Each core computes a local [20, 256] partial, AllReduce(sum) across the
8 cores, then every core normalizes columns and writes the full output.

v5 schedule. Facts this design is built on (measured in CoreSim):
  - DMA transfers issued from different engine queues overlap freely;
    within one queue they chain at ~12.6us per [h,64f,w] tile. A DMA
    blocks its issuing engine's instruction stream for the WHOLE
    transfer, so ACT/Pool DMAs trade compute 1:1.
  - Engines execute their instruction stream in order; cross-engine
    chunk pipelines stall on the slowest producer. So each sumsq chunk
    uses at most two engines: the square engine and DVE (which owns
    the final reduce).
  - Tile deps are per-tile: the one-hot mask is kept as FOUR w-quarter
    tiles so matmuls start as soon as their own quarter is scaled.
  - bf16 squares + bf16 TensorTensor tree adds get the DVE 2x_1p mode;
    TensorReduce/TensorScalarPtr(stt) get no DVE perf modes.
  - Mixed-dtype matmul (bf16 lhsT x f32r rhs) is legal and runs at
    1 cycle/row for N=256.

Work layout:
  SP  : lab0 lab1 b0fc0 b0fc3 b1fc0 b1fc1 out        (DMA only)
  ACT : b0fc1 | squares (fc0,fc1 + half of fc3) | b1fc2 | b1 squares
  Pool: b0fc2 | 8 mask builds | fc2 squares+L0, fc3 half-squares,
        mask-mult share | b1 fc3 quarters (late) | ...
  DVE : trees + reduces + inv chain + mask-mult share + epilogue
  PE  : segment-sum matmuls (PSUM accumulates across both batches),
        optional warm-keeper fillers between the two bursts.
"""

import os

import numpy as np

import concourse.bass as bass
import concourse.bacc as bacc
from concourse import mybir
from concourse.bass_utils import run_bass_kernel_spmd
from concourse.tile import TileContext

F32 = mybir.dt.float32
F32R = mybir.dt.float32r
BF16 = mybir.dt.bfloat16
AX = mybir.AxisListType
OP = mybir.AluOpType
ACT_FN = mybir.ActivationFunctionType

NCORES = 8
B_TOT = 16
BPC = B_TOT // NCORES  # batches per core
F = 256
H = 128
W = 128
C = 20  # 19 known + void
FC = 64  # f-chunk per tile
NFC = F // FC
QW = 4  # w-columns packed per matmul (lhsT = [h, QW*CP])
CP = 32  # class block padded to PSUM partition alignment
NQ = 32  # matmul q-groups per f-chunk
NQF = 4  # fc3 quarter-f pieces
FQ = FC // NQF
WQ = W // 4  # mask quarter width

EPS2 = 1e-24  # matches max(norm, 1e-12) in the reference

_NO_CC = bool(int(os.environ.get("KERNEL_NO_CC", "0")))
N_FILL = int(os.environ.get("KERNEL_NFILL", "0"))  # PE warm-keeper matmuls


def build_nc():
    mm_dt = F32R
    nc = bacc.Bacc("TRN2", target_bir_lowering=False)

    feats = nc.declare_dram_parameter("feats", [BPC, F, H, W], mm_dt, isOutput=False)
    labs = nc.declare_dram_parameter("labs", [BPC, H, W], F32, isOutput=False)
    out_d = nc.declare_dram_parameter("out", [F, C - 1], F32, isOutput=True)

    cc_in = nc.dram_tensor("cc_in", [C, F], F32)
    cc_out = nc.dram_tensor("cc_out", [C, F], F32, addr_space="Shared")

    with TileContext(nc) as tc:
        with (
            tc.tile_pool(name="consts", bufs=1) as consts,
            tc.tile_pool(name="xp", bufs=5) as xp,
            tc.tile_pool(name="sqp", bufs=2) as sqp,
            tc.tile_pool(name="auxp", bufs=1) as auxp,
            tc.tile_pool(name="ssqp", bufs=1) as ssqp,
            tc.tile_pool(name="invp", bufs=1) as invp,
            tc.tile_pool(name="mp", bufs=1) as mp,
            tc.tile_pool(name="labp", bufs=2) as labp,
            tc.tile_pool(name="finp", bufs=1) as finp,
            tc.tile_pool(name="psum", bufs=1, space="PSUM") as psum,
        ):
            iota_i = consts.tile([H, CP], mybir.dt.int32)
            nc.gpsimd.iota(iota_i, pattern=[[1, CP]], base=0, channel_multiplier=0)
            iota_sb = consts.tile([H, CP], F32)
            nc.vector.tensor_copy(iota_sb, iota_i)
            eps_sb = consts.tile([H, 1], F32)
            nc.vector.memset(eps_sb, EPS2)
            # preload the act table that serves Sqrt+Square (scratch output
            # goes into the already-consumed iota_i tile)
            nc.scalar.activation(
                out=iota_i[:, 0:1].bitcast(F32), in_=eps_sb, func=ACT_FN.Sqrt
            )

            feats_ap = feats.ap()
            labs_ap = labs.ap()

            lab_sb = []
            for b in range(BPC):
                lt = labp.tile([H, W], F32, tag="lab", name="lab")
                nc.sync.dma_start(out=lt, in_=labs_ap[b])
                lab_sb.append(lt)

            psq = []
            for fc in range(NFC):
                psq_t = psum.tile(
                    [QW * CP, FC * QW], F32, tag=f"ps{fc}", name=f"ps{fc}"
                )
                psq.append(psq_t)

            xts = [[None] * NFC for _ in range(BPC)]
            hfw = [feats_ap[b].rearrange("f h w -> h f w") for b in range(BPC)]

            def alloc_tile(b, fc):
                t = xp.tile([H, FC, W], mm_dt, tag="xt", name="xt")
                xts[b][fc] = t
                return t

            # ring order: b0fc0..3 -> s0..3, b1fc0 -> s4, b1fc1..3 -> s0..2
            for b in range(BPC):
                for fc in range(NFC):
                    alloc_tile(b, fc)

            def dma_full(eng, b, fc):
                eng.dma_start(
                    out=xts[b][fc], in_=hfw[b][:, fc * FC : (fc + 1) * FC, :]
                )

            def dma_quarter(eng, b, qtr):
                t = xts[b][NFC - 1]
                f0 = qtr * FQ
                fbase = (NFC - 1) * FC
                eng.dma_start(
                    out=t[:, f0 : f0 + FQ, :],
                    in_=hfw[b][:, fbase + f0 : fbase + f0 + FQ, :],
                )

            # early DMAs
            dma_full(nc.sync, 0, 0)
            dma_full(nc.scalar, 0, 1)
            dma_full(nc.gpsimd, 0, 2)
            dma_quarter(nc.gpsimd, 0, 0)
            dma_quarter(nc.gpsimd, 0, 1)
            dma_quarter(nc.gpsimd, 0, 2)
            dma_quarter(nc.gpsimd, 0, 3)
            dma_full(nc.sync, 1, 0)
            # late b1fc1 on SP (waits on s0; SP has nothing to block)
            dma_full(nc.sync, 1, 1)

            # one-hot masks: 4 w-quarter tiles per batch, built once on Pool
            m_sb = [[None] * 4 for _ in range(BPC)]
            for b in range(BPC):
                for i in range(4):
                    mt = mp.tile(
                        [H, WQ, CP], mm_dt, tag=f"m{b}{i}", name=f"m{b}{i}"
                    )
                    nc.vector.tensor_tensor(
                        out=mt[:],
                        in0=bass.AP(
                            tensor=iota_sb[:].tensor,
                            offset=iota_sb[:].offset,
                            ap=[iota_sb[:].ap[0], [0, WQ], [1, CP]],
                        ),
                        in1=lab_sb[b][:, i * WQ : (i + 1) * WQ].to_broadcast(
                            [H, WQ, CP]
                        ),
                        op=OP.is_equal,
                    )
                    m_sb[b][i] = mt

            NPART = (NFC - 1) + NQF - 1  # fc0..2 + q0..2; q3 reuses slot 0
            ssq_t = [None] * BPC

            def sq_chunk(b, fc, f0, fsz, wc0, wsz, slot, sq_eng, tree_eng):
                ssq = ssq_t[b]
                sq = sqp.tile([H, FC * 32], BF16, tag="sq", name="sq")
                sqv = sq[:, 0 : fsz * wsz].rearrange("h (f w) -> h f w", w=wsz)
                src = xts[b][fc][:, f0 : f0 + fsz, wc0 : wc0 + wsz].bitcast(F32)
                if sq_eng == "act":
                    nc.scalar.activation(out=sqv, in_=src, func=ACT_FN.Square)
                elif sq_eng == "pool":
                    nc.scalar.activation(out=sqv, in_=src, func=ACT_FN.Square)
                else:
                    nc.vector.scalar_tensor_tensor(
                        out=sqv, in0=src, scalar=0.0, in1=src,
                        op0=OP.bypass, op1=OP.mult,
                    )
                cur, csz = sqv, fsz
                pi = 0
                while csz > 16:
                    h1 = csz // 2
                    if pi == 0:
                        nxt = auxp.tile(
                            [H, FC // 2 * 32], BF16, tag="aux0", name="aux0"
                        )
                        nv = nxt[:, 0 : h1 * wsz].rearrange(
                            "h (f w) -> h f w", w=wsz
                        )
                    else:
                        # L1 writes back into the sq tile's head (its inputs
                        # now live in aux)
                        nv = sq[:, 0 : h1 * wsz].rearrange(
                            "h (f w) -> h f w", w=wsz
                        )
                    eng = nc.vector
                    pi += 1
                    eng.tensor_tensor(
                        out=nv, in0=cur[:, 0:h1, :], in1=cur[:, h1:csz, :], op=OP.add
                    )
                    cur, csz = nv, h1
                with nc.allow_low_precision("bf16 sumsq partials, tol 2e-2"):
                    nc.vector.tensor_reduce(
                        out=ssq[:, wc0 : wc0 + wsz, slot],
                        in_=cur.rearrange("h f w -> h w f"),
                        axis=AX.X,
                        op=OP.add,
                    )

            def batch_compute(b):
                ssq_t[b] = ssqp.tile([H, W, NPART], BF16, tag="ssq", name="ssq")
                # fc0, fc1: ACT squares, all-DVE tree
                for fc in range(2):
                    for wc in range(4):
                        sq_chunk(b, fc, 0, FC, wc * 32, 32, fc, "act", "dve")
                # fc2: Pool squares + Pool L0, DVE L1+reduce
                for wc in range(4):
                    sq_chunk(b, 2, 0, FC, wc * 32, 32, 2, "act", "dve")
                # fc3 in quarter-f pieces (b0: one DMA; b1: 4 late DMAs):
                # squares split ACT/Pool, direct DVE reduce
                for qtr in range(NQF):
                    slot = NFC - 1 + qtr if qtr < NQF - 1 else 0
                    for wc in range(4):
                        sq_eng = "act" if wc < 2 else "pool"
                        sq_chunk(
                            b, NFC - 1, qtr * FQ, FQ, wc * 32, 32,
                            slot, sq_eng, "dve",
                        )
                    if qtr == NQF - 2:
                        p_early = invp.tile([H, W], BF16, tag="pe", name="pe")
                        with nc.allow_low_precision("bf16 sumsq partial sum"):
                            nc.vector.tensor_reduce(
                                out=p_early,
                                in_=ssq_t[b][:, :, 0:NPART],
                                axis=AX.X,
                                op=OP.add,
                            )
                # inv = 1/sqrt(total + eps)
                ptot = invp.tile([H, W], F32, tag="pt", name="pt")
                nc.vector.tensor_add(ptot, p_early, ssq_t[b][:, :, 0])
                nc.scalar.activation(
                    out=ptot, in_=ptot, func=ACT_FN.Sqrt, bias=eps_sb[:]
                )
                # reuse the label tile (dead after mask builds) for inv
                inv = lab_sb[b]
                nc.vector.reciprocal(out=inv, in_=ptot)

                # mask quarters *= inv; independent tiles so matmuls start
                # per-quarter. DVE does q0/q2, Pool q1/q3.
                for i in range(4):
                    eng = nc.vector
                    mt = m_sb[b][i]
                    eng.tensor_tensor(
                        out=mt[:],
                        in0=mt[:],
                        in1=inv[:, i * WQ : (i + 1) * WQ].to_broadcast([H, WQ, CP]),
                        op=OP.mult,
                    )

                # segment-sum matmuls
                for fc in range(NFC):
                    for q in range(NQ):
                        mt = m_sb[b][q // 8]
                        ql = q % 8
                        nc.tensor.matmul(
                            out=psq[fc],
                            lhsT=mt[:, ql * QW : (ql + 1) * QW, :].rearrange(
                                "h w c -> h (w c)"
                            ),
                            rhs=xts[b][fc][:, :, q * QW : (q + 1) * QW],
                            start=(b == 0 and q == 0),
                            stop=(b == BPC - 1 and q == NQ - 1),
                        )

            batch_compute(0)

            # late DMAs for b1, emitted after b0's compute so the issuing
            # engines reach them with slot waits (mostly) satisfied
            dma_full(nc.gpsimd, 1, 2)  # Pool: waits s1
            for qtr in range(NQF):  # Pool: waits s2
                dma_quarter(nc.gpsimd, 1, qtr)

            if N_FILL:
                pwarm = psum.tile([H, FC * QW], F32, tag="pwarm", name="pwarm")
                junk = bass.AP(
                    tensor=iota_sb[:].tensor,
                    offset=iota_sb[:].offset,
                    ap=[iota_sb[:].ap[0], [0, FC * QW // CP], [1, CP]],
                )
                for _ in range(N_FILL):
                    nc.tensor.matmul(
                        out=pwarm[0:1, :],
                        lhsT=iota_sb[:, 0:1],
                        rhs=junk,
                        start=True,
                        stop=True,
                        skip_group_check=True,
                    )

            batch_compute(1)

            # combine diagonal blocks:
            #   protos[c, fc*FC + f] = sum_wl psq[fc][wl*CP+c, f*QW+wl]
            protos_sb = finp.tile([C, F], F32, tag="protos", name="protos")
            for fc in range(NFC):
                pv = psq[fc][:].rearrange("m (f w) -> m f w", w=QW)
                dst = protos_sb[0:C, fc * FC : (fc + 1) * FC]
                nc.scalar.copy(out=dst, in_=pv[0:C, :, 0])
                for wl in range(1, QW):
                    nc.vector.tensor_add(dst, dst, pv[wl * CP : wl * CP + C, :, wl])
            if not _NO_CC:
                nc.sync.dma_start(out=cc_in.ap(), in_=protos_sb)
                nc.gpsimd.collective_compute(
                    "AllReduce",
                    OP.add,
                    ins=[cc_in.ap().opt()],
                    outs=[cc_out.ap().opt()],
                    replica_groups=[list(range(NCORES))],
                )
                red_sb = finp.tile([C, F], F32, tag="red", name="red")
                nc.sync.dma_start(out=red_sb, in_=cc_out.ap())
            else:
                red_sb = protos_sb

            # column norms, fused square+sum via TSP accum
            scr_t = auxp.tile([H, FC // 2 * 32], BF16, tag="aux0", name="aux0")
            scr = scr_t[0:C, 0:F].rearrange("h (f w) -> h f w", w=1)[:, :, 0]
            pn = invp.tile([C, 1], F32, tag="pn", name="pn")
            nc.vector.scalar_tensor_tensor(
                out=scr,
                in0=red_sb,
                scalar=0.0,
                in1=red_sb,
                op0=OP.bypass,
                op1=OP.mult,
                accum_out=pn,
            )
            nc.scalar.activation(out=pn, in_=pn, func=ACT_FN.Sqrt, bias=eps_sb[0:C])
            pninv = invp.tile([C, 1], F32, tag="pninv", name="pninv")
            nc.vector.reciprocal(out=pninv, in_=pn)
            nc.vector.tensor_scalar_mul(out=red_sb, in0=red_sb, scalar1=pninv)

            # transposed write: out[f, c] = red_sb[c, f]
            o_ap = out_d.ap()
            nc.sync.dma_start(
                out=bass.AP(
                    tensor=o_ap.tensor,
                    offset=o_ap.offset,
                    ap=[[1, C - 1], [C - 1, F]],
                ),
                in_=red_sb[0 : C - 1, :],
            )

    nc.compile()
    return nc


_NC_CACHE = None


def _get_nc():
    global _NC_CACHE
    if _NC_CACHE is None:
        _NC_CACHE = build_nc()
    return _NC_CACHE


def kernel(features: np.ndarray, labels: np.ndarray) -> np.ndarray:
    features = np.ascontiguousarray(np.asarray(features, dtype=np.float32))
    labs_f32 = np.asarray(labels, dtype=np.float32)  # values 0..19, exact in f32

    nc = _get_nc()
    in_maps = []
    for core in range(NCORES):
        in_maps.append(
            {
                "feats": features[core * BPC : (core + 1) * BPC],
                "labs": np.ascontiguousarray(labs_f32[core * BPC : (core + 1) * BPC]),
            }
        )
    res = run_bass_kernel_spmd(nc, in_maps, core_ids=list(range(NCORES)))
    return np.asarray(res.results[0]["out"], dtype=np.float32)


# revision 26
# speedup vs baseline: 1.6473x; 1.0406x over previous
"""Trainium2 Bass kernel for nn_Extract_HyperSpherePrototypes.

Computation (see reference):
  1. L2-normalize each pixel's feature vector over the channel dim F=256.
  2. Segment-sum normalized features by label into [C+1=20, F] prototypes.
  3. Drop void class, transpose to [F, 19], L2-normalize each column.

Sharding: data-parallel over batch (16 items / 8 cores = 2 per core).
Each core computes a local [20, 256] partial, AllReduce(sum) across the
8 cores, then every core normalizes columns and writes the full output.

v5 schedule. Facts this design is built on (measured in CoreSim):
  - DMA transfers issued from different engine queues overlap freely;
    within one queue they chain at ~12.6us per [h,64f,w] tile. A DMA
    blocks its issuing engine's instruction stream for the WHOLE
    transfer, so ACT/Pool DMAs trade compute 1:1.
  - Engines execute their instruction stream in order; cross-engine
    chunk pipelines stall on the slowest producer. So each sumsq chunk
    uses at most two engines: the square engine and DVE (which owns
    the final reduce).
  - Tile deps are per-tile: the one-hot mask is kept as FOUR w-quarter
    tiles so matmuls start as soon as their own quarter is scaled.
  - bf16 squares + bf16 TensorTensor tree adds get the DVE 2x_1p mode;
    TensorReduce/TensorScalarPtr(stt) get no DVE perf modes.
  - Mixed-dtype matmul (bf16 lhsT x f32r rhs) is legal and runs at
    1 cycle/row for N=256.

Work layout:
  SP  : lab0 lab1 b0fc0 b0fc3 b1fc0 b1fc1 out        (DMA only)
  ACT : b0fc1 | squares (fc0,fc1 + half of fc3) | b1fc2 | b1 squares
  Pool: b0fc2 | 8 mask builds | fc2 squares+L0, fc3 half-squares,
        mask-mult share | b1 fc3 quarters (late) | ...
  DVE : trees + reduces + inv chain + mask-mult share + epilogue
  PE  : segment-sum matmuls (PSUM accumulates across both batches),
        optional warm-keeper fillers between the two bursts.
"""

import os

import numpy as np

import concourse.bass as bass
import concourse.bacc as bacc
from concourse import mybir
from concourse.bass_utils import run_bass_kernel_spmd
from concourse.tile import TileContext

F32 = mybir.dt.float32
F32R = mybir.dt.float32r
BF16 = mybir.dt.bfloat16
AX = mybir.AxisListType
OP = mybir.AluOpType
ACT_FN = mybir.ActivationFunctionType

NCORES = 8
B_TOT = 16
BPC = B_TOT // NCORES  # batches per core
F = 256
H = 128
W = 128
C = 20  # 19 known + void
FC = 64  # f-chunk per tile
NFC = F // FC
QW = 4  # w-columns packed per matmul (lhsT = [h, QW*CP])
CP = 32  # class block padded to PSUM partition alignment
NQ = 32  # matmul q-groups per f-chunk
NQF = 4  # fc3 quarter-f pieces
FQ = FC // NQF
WQ = W // 4  # mask quarter width

EPS2 = 1e-24  # matches max(norm, 1e-12) in the reference

_NO_CC = bool(int(os.environ.get("KERNEL_NO_CC", "0")))
N_FILL = int(os.environ.get("KERNEL_NFILL", "0"))  # PE warm-keeper matmuls


def build_nc():
    mm_dt = F32R
    nc = bacc.Bacc("TRN2", target_bir_lowering=False)

    feats = nc.declare_dram_parameter("feats", [BPC, F, H, W], mm_dt, isOutput=False)
    labs = nc.declare_dram_parameter("labs", [BPC, H, W], F32, isOutput=False)
    out_d = nc.declare_dram_parameter("out", [F, C - 1], F32, isOutput=True)

    cc_in = nc.dram_tensor("cc_in", [C, F], F32)
    cc_out = nc.dram_tensor("cc_out", [C, F], F32, addr_space="Shared")

    with TileContext(nc) as tc:
        with (
            tc.tile_pool(name="consts", bufs=1) as consts,
            tc.tile_pool(name="xp", bufs=5) as xp,
            tc.tile_pool(name="sqp", bufs=2) as sqp,
            tc.tile_pool(name="auxp", bufs=1) as auxp,
            tc.tile_pool(name="ssqp", bufs=1) as ssqp,
            tc.tile_pool(name="invp", bufs=1) as invp,
            tc.tile_pool(name="mp", bufs=1) as mp,
            tc.tile_pool(name="labp", bufs=2) as labp,
            tc.tile_pool(name="finp", bufs=1) as finp,
            tc.tile_pool(name="psum", bufs=1, space="PSUM") as psum,
        ):
            iota_i = consts.tile([H, CP], mybir.dt.int32)
            nc.gpsimd.iota(iota_i, pattern=[[1, CP]], base=0, channel_multiplier=0)
            iota_sb = consts.tile([H, CP], F32)
            nc.vector.tensor_copy(iota_sb, iota_i)
            eps_sb = consts.tile([H, 1], F32)
            nc.vector.memset(eps_sb, EPS2)
            # preload the act table that serves Sqrt+Square (scratch output
            # goes into the already-consumed iota_i tile)
            nc.scalar.activation(
                out=iota_i[:, 0:1].bitcast(F32), in_=eps_sb, func=ACT_FN.Sqrt
            )

            feats_ap = feats.ap()
            labs_ap = labs.ap()

            lab_sb = []
            for b in range(BPC):
                lt = labp.tile([H, W], F32, tag="lab", name="lab")
                nc.sync.dma_start(out=lt, in_=labs_ap[b])
                lab_sb.append(lt)

            psq = []
            for fc in range(NFC):
                psq_t = psum.tile(
                    [QW * CP, FC * QW], F32, tag=f"ps{fc}", name=f"ps{fc}"
                )
                psq.append(psq_t)

            xts = [[None] * NFC for _ in range(BPC)]
            hfw = [feats_ap[b].rearrange("f h w -> h f w") for b in range(BPC)]

            def alloc_tile(b, fc):
                t = xp.tile([H, FC, W], mm_dt, tag="xt", name="xt")
                xts[b][fc] = t
                return t

            # ring order: b0fc0..3 -> s0..3, b1fc0 -> s4, b1fc1..3 -> s0..2
            for b in range(BPC):
                for fc in range(NFC):
                    alloc_tile(b, fc)

            def dma_full(eng, b, fc):
                eng.dma_start(
                    out=xts[b][fc], in_=hfw[b][:, fc * FC : (fc + 1) * FC, :]
                )

            def dma_quarter(eng, b, qtr):
                t = xts[b][NFC - 1]
                f0 = qtr * FQ
                fbase = (NFC - 1) * FC
                eng.dma_start(
                    out=t[:, f0 : f0 + FQ, :],
                    in_=hfw[b][:, fbase + f0 : fbase + f0 + FQ, :],
                )

            # early DMAs
            dma_full(nc.sync, 0, 0)
            dma_full(nc.scalar, 0, 1)
            dma_full(nc.gpsimd, 0, 2)
            dma_quarter(nc.gpsimd, 0, 0)
            dma_quarter(nc.gpsimd, 0, 1)
            dma_quarter(nc.gpsimd, 0, 2)
            dma_quarter(nc.gpsimd, 0, 3)
            dma_full(nc.sync, 1, 0)
            # late b1fc1 on SP (waits on s0; SP has nothing to block)
            dma_full(nc.sync, 1, 1)

            # one-hot masks: 4 w-quarter tiles per batch, built once on Pool
            m_sb = [[None] * 4 for _ in range(BPC)]
            for b in range(BPC):
                for i in range(4):
                    mt = mp.tile(
                        [H, WQ, CP], mm_dt, tag=f"m{b}{i}", name=f"m{b}{i}"
                    )
                    nc.vector.tensor_tensor(
                        out=mt[:],
                        in0=bass.AP(
                            tensor=iota_sb[:].tensor,
                            offset=iota_sb[:].offset,
                            ap=[iota_sb[:].ap[0], [0, WQ], [1, CP]],
                        ),
                        in1=lab_sb[b][:, i * WQ : (i + 1) * WQ].to_broadcast(
                            [H, WQ, CP]
                        ),
                        op=OP.is_equal,
                    )
                    m_sb[b][i] = mt

            NPART = (NFC - 1) + NQF - 1  # fc0..2 + q0..2; q3 reuses slot 0
            ssq_t = [None] * BPC

            def sq_chunk(b, fc, f0, fsz, wc0, wsz, slot, sq_eng, tree_eng):
                ssq = ssq_t[b]
                sq = sqp.tile([H, FC * 32], BF16, tag="sq", name="sq")
                sqv = sq[:, 0 : fsz * wsz].rearrange("h (f w) -> h f w", w=wsz)
                src = xts[b][fc][:, f0 : f0 + fsz, wc0 : wc0 + wsz].bitcast(F32)
                if sq_eng == "act":
                    nc.scalar.activation(out=sqv, in_=src, func=ACT_FN.Square)
                elif sq_eng == "pool":
                    nc.scalar.activation(out=sqv, in_=src, func=ACT_FN.Square)
                else:
                    nc.vector.scalar_tensor_tensor(
                        out=sqv, in0=src, scalar=0.0, in1=src,
                        op0=OP.bypass, op1=OP.mult,
                    )
                cur, csz = sqv, fsz
                pi = 0
                while csz > 16:
                    h1 = csz // 2
                    if pi == 0:
                        nxt = auxp.tile(
                            [H, FC // 2 * 32], BF16, tag="aux0", name="aux0"
                        )
                        nv = nxt[:, 0 : h1 * wsz].rearrange(
                            "h (f w) -> h f w", w=wsz
                        )
                    else:
                        # L1 writes back into the sq tile's head (its inputs
                        # now live in aux)
                        nv = sq[:, 0 : h1 * wsz].rearrange(
                            "h (f w) -> h f w", w=wsz
                        )
                    eng = nc.vector
                    pi += 1
                    eng.tensor_tensor(
                        out=nv, in0=cur[:, 0:h1, :], in1=cur[:, h1:csz, :], op=OP.add
                    )
                    cur, csz = nv, h1
                with nc.allow_low_precision("bf16 sumsq partials, tol 2e-2"):
                    nc.vector.tensor_reduce(
                        out=ssq[:, wc0 : wc0 + wsz, slot],
                        in_=cur.rearrange("h f w -> h w f"),
                        axis=AX.X,
                        op=OP.add,
                    )

            def batch_compute(b):
                ssq_t[b] = ssqp.tile([H, W, NPART], BF16, tag="ssq", name="ssq")
                # fc0..2: ACT squares, all-DVE tree
                for fc in range(3):
                    for wc in range(4):
                        sq_chunk(b, fc, 0, FC, wc * 32, 32, fc, "act", "dve")
                # fc3: b0's pieces all arrive early -> one full 64f
                # chunk set (fewer, cheaper DVE trees); b1 merges q0+q1 and
                # keeps q2/q3 quarter-granular for the tail.
                # slots: fc0..2 -> 0..2; fc3 pieces -> 3..5 (b0 uses only 3)
                if b == 0:
                    pieces = [(0, FC, 3)]
                else:
                    pieces = [(0, 2 * FQ, 3), (2 * FQ, FQ, 4), (3 * FQ, FQ, 5)]
                last_slot = pieces[-1][2]
                pre_slots = 3 if b == 0 else 5

                def emit_p_early():
                    # combine everything except the last piece, off the tail
                    with nc.allow_low_precision("bf16 sumsq partial sum"):
                        nc.vector.tensor_reduce(
                            out=p_early,
                            in_=ssq_t[b][:, :, 0:pre_slots],
                            axis=AX.X,
                            op=OP.add,
                        )

                p_early = invp.tile([H, W], BF16, tag="pe", name="pe")
                if len(pieces) == 1:
                    emit_p_early()  # fc0..2 partials are already done
                for pi_, (f0, fsz, slot) in enumerate(pieces):
                    for wc in range(4):
                        sq_chunk(
                            b, NFC - 1, f0, fsz, wc * 32, 32,
                            slot, "act", "dve",
                        )
                    if len(pieces) > 1 and pi_ == len(pieces) - 2:
                        emit_p_early()
                # inv = 1/sqrt(total + eps)
                ptot = invp.tile([H, W], F32, tag="pt", name="pt")
                nc.vector.tensor_add(ptot, p_early, ssq_t[b][:, :, last_slot])
                nc.scalar.activation(
                    out=ptot, in_=ptot, func=ACT_FN.Sqrt, bias=eps_sb[:]
                )
                # reuse the label tile (dead after mask builds) for inv
                inv = lab_sb[b]
                nc.vector.reciprocal(out=inv, in_=ptot)

                # mask quarters *= inv; independent tiles so matmuls start
                # per-quarter. DVE does q0/q2, Pool q1/q3.
                for i in range(4):
                    eng = nc.vector
                    mt = m_sb[b][i]
                    eng.tensor_tensor(
                        out=mt[:],
                        in0=mt[:],
                        in1=inv[:, i * WQ : (i + 1) * WQ].to_broadcast([H, WQ, CP]),
                        op=OP.mult,
                    )

                # segment-sum matmuls
                for fc in range(NFC):
                    for q in range(NQ):
                        mt = m_sb[b][q // 8]
                        ql = q % 8
                        nc.tensor.matmul(
                            out=psq[fc],
                            lhsT=mt[:, ql * QW : (ql + 1) * QW, :].rearrange(
                                "h w c -> h (w c)"
                            ),
                            rhs=xts[b][fc][:, :, q * QW : (q + 1) * QW],
                            start=(b == 0 and q == 0),
                            stop=(b == BPC - 1 and q == NQ - 1),
                        )

            batch_compute(0)

            # late DMAs for b1, emitted after b0's compute so the issuing
            # engines reach them with slot waits (mostly) satisfied
            dma_full(nc.gpsimd, 1, 2)  # Pool: waits s1
            # fc3 quarters split across SP and Pool so the last piece lands
            # ~6us earlier than a serial Pool chain
            dma_quarter(nc.sync, 1, 0)
            dma_quarter(nc.sync, 1, 1)
            dma_quarter(nc.gpsimd, 1, 2)
            dma_quarter(nc.gpsimd, 1, 3)

            if N_FILL:
                pwarm = psum.tile([H, FC * QW], F32, tag="pwarm", name="pwarm")
                junk = bass.AP(
                    tensor=iota_sb[:].tensor,
                    offset=iota_sb[:].offset,
                    ap=[iota_sb[:].ap[0], [0, FC * QW // CP], [1, CP]],
                )
                for _ in range(N_FILL):
                    nc.tensor.matmul(
                        out=pwarm[0:1, :],
                        lhsT=iota_sb[:, 0:1],
                        rhs=junk,
                        start=True,
                        stop=True,
                        skip_group_check=True,
                    )

            batch_compute(1)

            # combine diagonal blocks:
            #   protos[c, fc*FC + f] = sum_wl psq[fc][wl*CP+c, f*QW+wl]
            protos_sb = finp.tile([C, F], F32, tag="protos", name="protos")
            for fc in range(NFC):
                pv = psq[fc][:].rearrange("m (f w) -> m f w", w=QW)
                dst = protos_sb[0:C, fc * FC : (fc + 1) * FC]
                nc.scalar.copy(out=dst, in_=pv[0:C, :, 0])
                for wl in range(1, QW):
                    nc.vector.tensor_add(dst, dst, pv[wl * CP : wl * CP + C, :, wl])
            if not _NO_CC:
                nc.sync.dma_start(out=cc_in.ap(), in_=protos_sb)
                nc.gpsimd.collective_compute(
                    "AllReduce",
                    OP.add,
                    ins=[cc_in.ap().opt()],
                    outs=[cc_out.ap().opt()],
                    replica_groups=[list(range(NCORES))],
                )
                red_sb = finp.tile([C, F], F32, tag="red", name="red")
                nc.sync.dma_start(out=red_sb, in_=cc_out.ap())
            else:
                red_sb = protos_sb

            # column norms, fused square+sum via TSP accum
            scr_t = auxp.tile([H, FC // 2 * 32], BF16, tag="aux0", name="aux0")
            scr = scr_t[0:C, 0:F].rearrange("h (f w) -> h f w", w=1)[:, :, 0]
            pn = invp.tile([C, 1], F32, tag="pn", name="pn")
            nc.vector.scalar_tensor_tensor(
                out=scr,
                in0=red_sb,
                scalar=0.0,
                in1=red_sb,
                op0=OP.bypass,
                op1=OP.mult,
                accum_out=pn,
            )
            nc.scalar.activation(out=pn, in_=pn, func=ACT_FN.Sqrt, bias=eps_sb[0:C])
            pninv = invp.tile([C, 1], F32, tag="pninv", name="pninv")
            nc.vector.reciprocal(out=pninv, in_=pn)
            nc.vector.tensor_scalar_mul(out=red_sb, in0=red_sb, scalar1=pninv)

            # transposed write: out[f, c] = red_sb[c, f]
            o_ap = out_d.ap()
            nc.sync.dma_start(
                out=bass.AP(
                    tensor=o_ap.tensor,
                    offset=o_ap.offset,
                    ap=[[1, C - 1], [C - 1, F]],
                ),
                in_=red_sb[0 : C - 1, :],
            )

    nc.compile()
    return nc


_NC_CACHE = None


def _get_nc():
    global _NC_CACHE
    if _NC_CACHE is None:
        _NC_CACHE = build_nc()
    return _NC_CACHE


def kernel(features: np.ndarray, labels: np.ndarray) -> np.ndarray:
    features = np.ascontiguousarray(np.asarray(features, dtype=np.float32))
    labs_f32 = np.asarray(labels, dtype=np.float32)  # values 0..19, exact in f32

    nc = _get_nc()
    in_maps = []
    for core in range(NCORES):
        in_maps.append(
            {
                "feats": features[core * BPC : (core + 1) * BPC],
                "labs": np.ascontiguousarray(labs_f32[core * BPC : (core + 1) * BPC]),
            }
        )
    res = run_bass_kernel_spmd(nc, in_maps, core_ids=list(range(NCORES)))
    return np.asarray(res.results[0]["out"], dtype=np.float32)


# revision 30
# speedup vs baseline: 1.6543x; 1.0042x over previous
"""Trainium2 Bass kernel for nn_Extract_HyperSpherePrototypes.

Computation (see reference):
  1. L2-normalize each pixel's feature vector over the channel dim F=256.
  2. Segment-sum normalized features by label into [C+1=20, F] prototypes.
  3. Drop void class, transpose to [F, 19], L2-normalize each column.

Sharding: data-parallel over batch (16 items / 8 cores = 2 per core).
Each core computes a local [20, 256] partial, AllReduce(sum) across the
8 cores, then every core normalizes columns and writes the full output.

Schedule facts this design is built on (measured in CoreSim, verified
against the neuronxcc hardware path):
  - DMA transfers issued from different engine queues (SP HWDGE, ACT
    HWDGE, Pool SWDGE) overlap freely; within one queue they chain at
    ~12.6us per [h,64f,w] tile. A DMA occupies its issuing engine for
    the whole transfer, so ACT/Pool DMA trades compute 1:1.
  - On real hardware Pool/GPSIMD cannot run TensorTensor (NCC_IXCG966)
    and matmul may not mix 32-bit with 16-bit inputs (NCC_IBIR034) --
    CoreSim permits both. So Pool is used ONLY for DMA/iota/collective
    and masks are f32r.
  - Engines execute in order (with a 4-deep wait queue that lets ready
    instructions pass waiting ones), so each sumsq chunk uses only ACT
    (square, bf16 out) + DVE (bf16 TensorTensor add tree with the
    2x_1p mode + one small TensorReduce).
  - Tile deps are tile-granular: the one-hot mask is FOUR w-quarter
    tiles per batch so matmuls start per-quarter right after inv.

Work layout (b0 = early batch, b1 = late batch gated on xt slots):
  SP  : lab0 lab1 b0fc0 b1fc0 | b1fc1 b1q0 b1q1 | out
  ACT : b0fc1 | all squares (bf16) | sqrt
  Pool: b0fc2 b0q0..3 | b1fc2 b1q2 b1q3            (DMA only)
  DVE : mask builds, add-trees + reduces, inv chain, mask*inv,
        PSUM diag combine + column-norm epilogue
  PE  : segment-sum matmuls, f32r rhs at N=256 = 1 cycle/row,
        PSUM accumulates across both batches.
b0's fc3 is processed as full 64f chunks (cheap trees); b1's fc3 is
DMA'd as quarter-f pieces (q0+q1 merged, q2/q3 separate) so the
norm -> mask -> matmul tail starts as early as possible.
"""

import os

import numpy as np

import concourse.bass as bass
import concourse.bacc as bacc
from concourse import mybir
from concourse.bass_utils import run_bass_kernel_spmd
from concourse.tile import TileContext

F32 = mybir.dt.float32
F32R = mybir.dt.float32r
BF16 = mybir.dt.bfloat16
AX = mybir.AxisListType
OP = mybir.AluOpType
ACT_FN = mybir.ActivationFunctionType

NCORES = 8
B_TOT = 16
BPC = B_TOT // NCORES  # batches per core
F = 256
H = 128
W = 128
C = 20  # 19 known + void
FC = 64  # f-chunk per tile
NFC = F // FC
QW = 4  # w-columns packed per matmul (lhsT = [h, QW*CP])
CP = 32  # class block padded to PSUM partition alignment
NQ = 32  # matmul q-groups per f-chunk
NQF = 4  # fc3 quarter-f pieces
FQ = FC // NQF
WQ = W // 4  # mask quarter width

EPS2 = 1e-24  # matches max(norm, 1e-12) in the reference

_NO_CC = bool(int(os.environ.get("KERNEL_NO_CC", "0")))
N_FILL = int(os.environ.get("KERNEL_NFILL", "0"))  # PE warm-keeper matmuls


def build_nc():
    mm_dt = F32R
    nc = bacc.Bacc("TRN2", target_bir_lowering=False)

    feats = nc.declare_dram_parameter("feats", [BPC, F, H, W], mm_dt, isOutput=False)
    labs = nc.declare_dram_parameter("labs", [BPC, H, W], F32, isOutput=False)
    out_d = nc.declare_dram_parameter("out", [F, C - 1], F32, isOutput=True)

    cc_in = nc.dram_tensor("cc_in", [C, F], F32)
    cc_out = nc.dram_tensor("cc_out", [C, F], F32, addr_space="Shared")

    with TileContext(nc) as tc:
        with (
            tc.tile_pool(name="consts", bufs=1) as consts,
            tc.tile_pool(name="xp", bufs=5) as xp,
            tc.tile_pool(name="sqp", bufs=3) as sqp,
            tc.tile_pool(name="auxp", bufs=1) as auxp,
            tc.tile_pool(name="ssqp", bufs=1) as ssqp,
            tc.tile_pool(name="invp", bufs=1) as invp,
            tc.tile_pool(name="mp", bufs=1) as mp,
            tc.tile_pool(name="labp", bufs=2) as labp,
            tc.tile_pool(name="finp", bufs=1) as finp,
            tc.tile_pool(name="psum", bufs=1, space="PSUM") as psum,
        ):
            iota_i = consts.tile([H, CP], mybir.dt.int32)
            nc.gpsimd.iota(iota_i, pattern=[[1, CP]], base=0, channel_multiplier=0)
            iota_sb = consts.tile([H, CP], F32)
            nc.vector.tensor_copy(iota_sb, iota_i)
            eps_sb = consts.tile([H, 1], F32)
            nc.vector.memset(eps_sb, EPS2)
            # preload the act table that serves Sqrt+Square (scratch output
            # goes into the already-consumed iota_i tile)
            nc.scalar.activation(
                out=iota_i[:, 0:1].bitcast(F32), in_=eps_sb, func=ACT_FN.Sqrt
            )

            feats_ap = feats.ap()
            labs_ap = labs.ap()

            lab_sb = []
            for b in range(BPC):
                lt = labp.tile([H, W], F32, tag="lab", name="lab")
                nc.sync.dma_start(out=lt, in_=labs_ap[b])
                lab_sb.append(lt)

            psq = []
            for fc in range(NFC):
                psq_t = psum.tile(
                    [QW * CP, FC * QW], F32, tag=f"ps{fc}", name=f"ps{fc}"
                )
                psq.append(psq_t)

            xts = [[None] * NFC for _ in range(BPC)]
            hfw = [feats_ap[b].rearrange("f h w -> h f w") for b in range(BPC)]

            def alloc_tile(b, fc):
                t = xp.tile([H, FC, W], mm_dt, tag="xt", name="xt")
                xts[b][fc] = t
                return t

            # ring order: b0fc0..3 -> s0..3, b1fc0 -> s4, b1fc1..3 -> s0..2
            for b in range(BPC):
                for fc in range(NFC):
                    alloc_tile(b, fc)

            def dma_full(eng, b, fc):
                eng.dma_start(
                    out=xts[b][fc], in_=hfw[b][:, fc * FC : (fc + 1) * FC, :]
                )

            def dma_quarter(eng, b, qtr):
                t = xts[b][NFC - 1]
                f0 = qtr * FQ
                fbase = (NFC - 1) * FC
                eng.dma_start(
                    out=t[:, f0 : f0 + FQ, :],
                    in_=hfw[b][:, fbase + f0 : fbase + f0 + FQ, :],
                )

            # early DMAs
            dma_full(nc.sync, 0, 0)
            dma_full(nc.scalar, 0, 1)
            dma_full(nc.gpsimd, 0, 2)
            dma_quarter(nc.gpsimd, 0, 0)
            dma_quarter(nc.gpsimd, 0, 1)
            dma_quarter(nc.gpsimd, 0, 2)
            dma_quarter(nc.gpsimd, 0, 3)
            dma_full(nc.sync, 1, 0)
            # late b1fc1 on SP (waits on s0; SP has nothing to block)
            dma_full(nc.sync, 1, 1)

            # one-hot masks: 4 w-quarter tiles per batch. b1's q3 shares
            # b0's q0 buffer (built late, once b0's matmuls release it) so
            # the 4KB saved buys a third square-scratch buffer.
            m_sb = [[None] * 4 for _ in range(BPC)]

            def build_mask(b, i, tag):
                mt = mp.tile([H, WQ, CP], mm_dt, tag=tag, name=f"m{b}{i}")
                nc.vector.tensor_tensor(
                    out=mt[:],
                    in0=bass.AP(
                        tensor=iota_sb[:].tensor,
                        offset=iota_sb[:].offset,
                        ap=[iota_sb[:].ap[0], [0, WQ], [1, CP]],
                    ),
                    in1=lab_sb[b][:, i * WQ : (i + 1) * WQ].to_broadcast(
                        [H, WQ, CP]
                    ),
                    op=OP.is_equal,
                )
                m_sb[b][i] = mt

            for b in range(BPC):
                for i in range(4):
                    if b == 1 and i == 3:
                        continue
                    build_mask(b, i, f"m{b}{i}")

            NPART = (NFC - 1) + NQF - 1  # fc0..2 + q0..2; q3 reuses slot 0
            ssq_t = [None] * BPC

            def sq_chunk(b, fc, f0, fsz, wc0, wsz, slot, sq_eng, tree_eng):
                ssq = ssq_t[b]
                sq = sqp.tile([H, FC * 32], BF16, tag="sq", name="sq")
                sqv = sq[:, 0 : fsz * wsz].rearrange("h (f w) -> h f w", w=wsz)
                src = xts[b][fc][:, f0 : f0 + fsz, wc0 : wc0 + wsz].bitcast(F32)
                if sq_eng == "act":
                    nc.scalar.activation(out=sqv, in_=src, func=ACT_FN.Square)
                elif sq_eng == "pool":
                    nc.scalar.activation(out=sqv, in_=src, func=ACT_FN.Square)
                else:
                    nc.vector.scalar_tensor_tensor(
                        out=sqv, in0=src, scalar=0.0, in1=src,
                        op0=OP.bypass, op1=OP.mult,
                    )
                cur, csz = sqv, fsz
                pi = 0
                while csz > 16:
                    h1 = csz // 2
                    if pi == 0:
                        nxt = auxp.tile(
                            [H, FC // 2 * 32], BF16, tag="aux0", name="aux0"
                        )
                        nv = nxt[:, 0 : h1 * wsz].rearrange(
                            "h (f w) -> h f w", w=wsz
                        )
                    else:
                        # L1 writes back into the sq tile's head (its inputs
                        # now live in aux)
                        nv = sq[:, 0 : h1 * wsz].rearrange(
                            "h (f w) -> h f w", w=wsz
                        )
                    eng = nc.vector
                    pi += 1
                    eng.tensor_tensor(
                        out=nv, in0=cur[:, 0:h1, :], in1=cur[:, h1:csz, :], op=OP.add
                    )
                    cur, csz = nv, h1
                with nc.allow_low_precision("bf16 sumsq partials, tol 2e-2"):
                    nc.vector.tensor_reduce(
                        out=ssq[:, wc0 : wc0 + wsz, slot],
                        in_=cur.rearrange("h f w -> h w f"),
                        axis=AX.X,
                        op=OP.add,
                    )

            def batch_compute(b):
                ssq_t[b] = ssqp.tile([H, W, NPART], BF16, tag="ssq", name="ssq")
                # fc0..2: ACT squares, all-DVE tree
                for fc in range(3):
                    for wc in range(4):
                        sq_chunk(b, fc, 0, FC, wc * 32, 32, fc, "act", "dve")
                # fc3: b0's pieces all arrive early -> one full 64f
                # chunk set (fewer, cheaper DVE trees); b1 merges q0+q1 and
                # keeps q2/q3 quarter-granular for the tail.
                # slots: fc0..2 -> 0..2; fc3 pieces -> 3..5 (b0 uses only 3)
                if b == 0:
                    pieces = [(0, FC, 3)]
                else:
                    pieces = [(0, 2 * FQ, 3), (2 * FQ, FQ, 4), (3 * FQ, FQ, 5)]
                last_slot = pieces[-1][2]
                pre_slots = 3 if b == 0 else 5

                def emit_p_early():
                    # combine everything except the last piece, off the tail
                    with nc.allow_low_precision("bf16 sumsq partial sum"):
                        nc.vector.tensor_reduce(
                            out=p_early,
                            in_=ssq_t[b][:, :, 0:pre_slots],
                            axis=AX.X,
                            op=OP.add,
                        )

                p_early = invp.tile([H, W], BF16, tag="pe", name="pe")
                if len(pieces) == 1:
                    emit_p_early()  # fc0..2 partials are already done
                for pi_, (f0, fsz, slot) in enumerate(pieces):
                    for wc in range(4):
                        sq_chunk(
                            b, NFC - 1, f0, fsz, wc * 32, 32,
                            slot, "act", "dve",
                        )
                    if len(pieces) > 1 and pi_ == len(pieces) - 2:
                        emit_p_early()
                # inv = 1/sqrt(total + eps)
                ptot = invp.tile([H, W], F32, tag="pt", name="pt")
                nc.vector.tensor_add(ptot, p_early, ssq_t[b][:, :, last_slot])
                nc.scalar.activation(
                    out=ptot, in_=ptot, func=ACT_FN.Sqrt, bias=eps_sb[:]
                )
                # reuse the label tile (dead after mask builds) for inv
                inv = lab_sb[b]
                nc.vector.reciprocal(out=inv, in_=ptot)

                # mask quarters *= inv; independent tiles so matmuls start
                # per-quarter. DVE does q0/q2, Pool q1/q3.
                for i in range(4):
                    eng = nc.vector
                    mt = m_sb[b][i]
                    eng.tensor_tensor(
                        out=mt[:],
                        in0=mt[:],
                        in1=inv[:, i * WQ : (i + 1) * WQ].to_broadcast([H, WQ, CP]),
                        op=OP.mult,
                    )

                # segment-sum matmuls
                for fc in range(NFC):
                    for q in range(NQ):
                        mt = m_sb[b][q // 8]
                        ql = q % 8
                        nc.tensor.matmul(
                            out=psq[fc],
                            lhsT=mt[:, ql * QW : (ql + 1) * QW, :].rearrange(
                                "h w c -> h (w c)"
                            ),
                            rhs=xts[b][fc][:, :, q * QW : (q + 1) * QW],
                            start=(b == 0 and q == 0),
                            stop=(b == BPC - 1 and q == NQ - 1),
                        )

            batch_compute(0)

            # b1's last mask quarter, into b0-q0's buffer (free after b0's
            # fc3 q0..7 matmuls)
            build_mask(1, 3, "m00")

            # late DMAs for b1, emitted after b0's compute so the issuing
            # engines reach them with slot waits (mostly) satisfied
            dma_full(nc.gpsimd, 1, 2)  # Pool: waits s1
            # fc3 quarters split across SP and Pool so the last piece lands
            # ~6us earlier than a serial Pool chain
            dma_quarter(nc.sync, 1, 0)
            dma_quarter(nc.sync, 1, 1)
            dma_quarter(nc.gpsimd, 1, 2)
            dma_quarter(nc.gpsimd, 1, 3)

            if N_FILL:
                pwarm = psum.tile([H, FC * QW], F32, tag="pwarm", name="pwarm")
                junk = bass.AP(
                    tensor=iota_sb[:].tensor,
                    offset=iota_sb[:].offset,
                    ap=[iota_sb[:].ap[0], [0, FC * QW // CP], [1, CP]],
                )
                for _ in range(N_FILL):
                    nc.tensor.matmul(
                        out=pwarm[0:1, :],
                        lhsT=iota_sb[:, 0:1],
                        rhs=junk,
                        start=True,
                        stop=True,
                        skip_group_check=True,
                    )

            batch_compute(1)

            # combine diagonal blocks:
            #   protos[c, fc*FC + f] = sum_wl psq[fc][wl*CP+c, f*QW+wl]
            protos_sb = finp.tile([C, F], F32, tag="protos", name="protos")
            for fc in range(NFC):
                pv = psq[fc][:].rearrange("m (f w) -> m f w", w=QW)
                dst = protos_sb[0:C, fc * FC : (fc + 1) * FC]
                nc.scalar.copy(out=dst, in_=pv[0:C, :, 0])
                for wl in range(1, QW):
                    nc.vector.tensor_add(dst, dst, pv[wl * CP : wl * CP + C, :, wl])
            if not _NO_CC:
                nc.sync.dma_start(out=cc_in.ap(), in_=protos_sb)
                nc.gpsimd.collective_compute(
                    "AllReduce",
                    OP.add,
                    ins=[cc_in.ap().opt()],
                    outs=[cc_out.ap().opt()],
                    replica_groups=[list(range(NCORES))],
                )
                red_sb = finp.tile([C, F], F32, tag="red", name="red")
                nc.sync.dma_start(out=red_sb, in_=cc_out.ap())
            else:
                red_sb = protos_sb

            # column norms, fused square+sum via TSP accum
            scr_t = auxp.tile([H, FC // 2 * 32], BF16, tag="aux0", name="aux0")
            scr = scr_t[0:C, 0:F].rearrange("h (f w) -> h f w", w=1)[:, :, 0]
            pn = invp.tile([C, 1], F32, tag="pn", name="pn")
            nc.vector.scalar_tensor_tensor(
                out=scr,
                in0=red_sb,
                scalar=0.0,
                in1=red_sb,
                op0=OP.bypass,
                op1=OP.mult,
                accum_out=pn,
            )
            nc.scalar.activation(out=pn, in_=pn, func=ACT_FN.Sqrt, bias=eps_sb[0:C])
            pninv = invp.tile([C, 1], F32, tag="pninv", name="pninv")
            nc.vector.reciprocal(out=pninv, in_=pn)
            nc.vector.tensor_scalar_mul(out=red_sb, in0=red_sb, scalar1=pninv)

            # transposed write: out[f, c] = red_sb[c, f]
            o_ap = out_d.ap()
            nc.sync.dma_start(
                out=bass.AP(
                    tensor=o_ap.tensor,
                    offset=o_ap.offset,
                    ap=[[1, C - 1], [C - 1, F]],
                ),
                in_=red_sb[0 : C - 1, :],
            )

    nc.compile()
    return nc


_NC_CACHE = None


def _get_nc():
    global _NC_CACHE
    if _NC_CACHE is None:
        _NC_CACHE = build_nc()
    return _NC_CACHE


def kernel(features: np.ndarray, labels: np.ndarray) -> np.ndarray:
    features = np.ascontiguousarray(np.asarray(features, dtype=np.float32))
    labs_f32 = np.asarray(labels, dtype=np.float32)  # values 0..19, exact in f32

    nc = _get_nc()
    in_maps = []
    for core in range(NCORES):
        in_maps.append(
            {
                "feats": features[core * BPC : (core + 1) * BPC],
                "labs": np.ascontiguousarray(labs_f32[core * BPC : (core + 1) * BPC]),
            }
        )
    res = run_bass_kernel_spmd(nc, in_maps, core_ids=list(range(NCORES)))
    return np.asarray(res.results[0]["out"], dtype=np.float32)
